# revision 8
# baseline (speedup 1.0000x reference)
"""Trainium2 Bass kernel for nn_DifferentiableColorMLPRenderer.

Sharding: data-parallel over the batch — core b renders image b (B=8 images,
8 NeuronCores). The tiny MLP weights and (host-joined) per-face feature table
are replicated to every core.

Per-core device pipeline (512x512 pixels, partition p owns pixels
[p*2048, (p+1)*2048)):
  - per 128-pixel block: indirect-DMA gather of the 36-byte per-face feature
    row G[pix_to_face] from HBM (G = feature[faces] joined on host, O(F) prep)
  - DVE: t = G * bary (free-dim broadcast AP) + mask cols -> [128, 32*4] tile
    (per block: 4 mask cols + 9 t cols, padded to a 32-col stanza)
  - PE transpose -> PSUM [128, 128] -> partition-remap copies -> rhs [13, 512]
  - L1 matmul K=13 (mask rows hit zero weights), Relu+bias on ACT
  - L2 matmul 128x128, Relu+bias
  - L3 matmul 128->4 (4th column zero), Relu+bias
  Matmuls run in float32r (1 cycle/row vs 4 for fp32; ~2e-4 rel err,
  tf32-class). Everything else stays exact fp32.
  - mask fold: out = (relu - [2,2,2,-1]) * mask + [1,1,1,0]
    (rows 0-2: where(mask, relu-1, 1); row 3: the mask channel)
  - PE transpose of each [4,128] block to pixel-major [128,4], staged per
    group and stored with one DMA of 64B-contiguous runs per partition
"""

import numpy as np
import jax
import concourse.bacc as bacc
import concourse.bass as bass
import concourse.mybir as mybir
from concourse.tile import TileContext
from concourse.masks import make_identity
from concourse import bass2jax
from concourse.bass2jax import _bass_exec_p, install_neuronx_cc_hook, partition_id_tensor
from jax.sharding import Mesh, NamedSharding, PartitionSpec
from jax.experimental.shard_map import shard_map

B, H, W = 8, 512, 512
V, F = 50000, 100000
P = 128
COLS = (H * W) // P  # 2048 pixels per partition
GB = 4               # blocks per group -> N=512 pixel tiles
GT_BUFS = 32

_CACHE = {}


def _build_kernel(cols=COLS, gb=GB, n_cores=B, gt_bufs=GT_BUFS):
    npix = P * cols
    ngroups = cols // gb
    N = gb * P

    nc = bacc.Bacc("TRN2", target_bir_lowering=False, debug=False,
                   num_devices=n_cores)
    dt = mybir.dt
    pf = nc.dram_tensor("pf", [P, cols], dt.int32, kind="ExternalInput")
    bary = nc.dram_tensor("bary", [P, cols * 3], dt.float32, kind="ExternalInput")
    gtab = nc.dram_tensor("gtab", [F, 9], dt.float32, kind="ExternalInput")
    w1e = nc.dram_tensor("w1e", [13, 128], dt.float32, kind="ExternalInput")
    b1 = nc.dram_tensor("b1c", [128, 1], dt.float32, kind="ExternalInput")
    w2 = nc.dram_tensor("w2", [128, 128], dt.float32, kind="ExternalInput")
    b2 = nc.dram_tensor("b2c", [128, 1], dt.float32, kind="ExternalInput")
    w3 = nc.dram_tensor("w3", [128, 4], dt.float32, kind="ExternalInput")
    b3 = nc.dram_tensor("b3c", [4, 1], dt.float32, kind="ExternalInput")
    sub4 = nc.dram_tensor("sub4", [4, 1], dt.float32, kind="ExternalInput")  # holds -[2,2,2,-1]
    add4 = nc.dram_tensor("add4", [4, 1], dt.float32, kind="ExternalInput")
    out = nc.dram_tensor("out", [npix, 4], dt.float32, kind="ExternalOutput")

    with TileContext(nc) as tc:
        with (
            tc.tile_pool(name="const", bufs=1) as cpool,
            tc.tile_pool(name="sbuf", bufs=2) as pool,
            tc.tile_pool(name="psum", bufs=2, space="PSUM") as ppool,
        ):
            ident = cpool.tile([P, P], dt.float32, tag="ident")
            make_identity(nc, ident[:])
            w1e_s = cpool.tile([13, 128], dt.float32, tag="w1e")
            nc.sync.dma_start(out=w1e_s[:], in_=w1e[:])
            w2_s = cpool.tile([128, 128], dt.float32, tag="w2")
            nc.sync.dma_start(out=w2_s[:], in_=w2[:])
            w3_s = cpool.tile([128, 4], dt.float32, tag="w3")
            nc.sync.dma_start(out=w3_s[:], in_=w3[:])
            b1_s = cpool.tile([128, 1], dt.float32, tag="b1")
            nc.sync.dma_start(out=b1_s[:], in_=b1[:])
            b2_s = cpool.tile([128, 1], dt.float32, tag="b2")
            nc.sync.dma_start(out=b2_s[:], in_=b2[:])
            b3_s = cpool.tile([4, 1], dt.float32, tag="b3")
            nc.sync.dma_start(out=b3_s[:], in_=b3[:])
            sub4_s = cpool.tile([4, 1], dt.float32, tag="sub4")
            nc.sync.dma_start(out=sub4_s[:], in_=sub4[:])
            w1e_r = cpool.tile([13, 128], dt.float32r, tag="w1er")
            nc.vector.tensor_copy(out=w1e_r[:], in_=w1e_s[:])
            w2_r = cpool.tile([128, 128], dt.float32r, tag="w2r")
            nc.vector.tensor_copy(out=w2_r[:], in_=w2_s[:])
            w3_r = cpool.tile([128, 4], dt.float32r, tag="w3r")
            nc.vector.tensor_copy(out=w3_r[:], in_=w3_s[:])
            add4_s = cpool.tile([4, 1], dt.float32, tag="add4")
            nc.sync.dma_start(out=add4_s[:], in_=add4[:])

            pf_s = cpool.tile([P, cols], dt.int32, tag="pf")
            nc.sync.dma_start(out=pf_s[:], in_=pf[:])
            bary_s = cpool.tile([P, cols * 3], dt.float32, tag="bary")
            nc.sync.dma_start(out=bary_s[:], in_=bary[:])
            maskf = cpool.tile([P, cols], dt.float32, tag="maskf")
            nc.vector.tensor_scalar(
                out=maskf[:], in0=pf_s[:], scalar1=0, scalar2=None,
                op0=mybir.AluOpType.is_gt)

            oview = out.ap().rearrange("(p c2) f -> p c2 f", p=P)

            # pre-memset rotating text buffers: the pad columns (13-31 of
            # each 32-col stanza) only ever hold finite stale data, and their
            # transposed PSUM rows are never read.
            n_text = 4
            texts = []
            for i in range(n_text):
                tx = pool.tile([P, 32 * gb], dt.float32, tag=f"text{i}",
                               bufs=1)
                nc.vector.memset(tx[:], 0)
                texts.append(tx)

            for g in range(ngroups):
                gt = pool.tile([P, gb * 9], dt.float32, tag="gt", bufs=gt_bufs)
                for j in range(gb):
                    k = g * gb + j
                    nc.gpsimd.indirect_dma_start(
                        out=gt[:, 9 * j:9 * j + 9],
                        out_offset=None,
                        in_=gtab[:],
                        in_offset=bass.IndirectOffsetOnAxis(
                            ap=pf_s[:, k:k + 1], axis=0),
                    )
                text = texts[g % n_text]
                bsl = bary_s[:, 3 * gb * g:3 * gb * (g + 1)]
                bary_bc = bass.AP(bsl.tensor, bsl.offset,
                                  [bsl.ap[0], bsl.ap[1], [0, 3]])
                t_view = bass.AP(text[:].tensor, text[:].offset + 4,
                                 [text[:].ap[0], [32, gb], [1, 9]])
                nc.vector.tensor_tensor(out=t_view, in0=gt[:].rearrange(
                    "p (b n) -> p b n", n=9), in1=bary_bc,
                    op=mybir.AluOpType.mult)
                msl = maskf[:, gb * g:gb * (g + 1)]
                m_bc = bass.AP(msl.tensor, msl.offset,
                               [msl.ap[0], msl.ap[1], [0, 4]])
                m_view = bass.AP(text[:].tensor, text[:].offset,
                                 [text[:].ap[0], [32, gb], [1, 4]])
                nc.scalar.copy(out=m_view, in_=m_bc)

                ptr = ppool.tile([32 * gb, P], dt.float32, tag="ptr", bufs=2)
                nc.tensor.transpose(out=ptr[:], in_=text[:], identity=ident[:])
                rhs = pool.tile([13, N], dt.float32r, tag="rhs", bufs=4)
                for j in range(gb):
                    nc.vector.tensor_copy(out=rhs[:, P * j:P * (j + 1)],
                                          in_=ptr[32 * j:32 * j + 13, :])

                p1 = ppool.tile([128, N], dt.float32, tag="p1", bufs=2)
                nc.tensor.matmul(out=p1[:], lhsT=w1e_r[:], rhs=rhs[:],
                                 start=True, stop=True)
                h1 = pool.tile([128, N], dt.float32r, tag="h1", bufs=3)
                nc.scalar.activation(h1[:], p1[:],
                                     mybir.ActivationFunctionType.Relu,
                                     bias=b1_s[:])
                p2 = ppool.tile([128, N], dt.float32, tag="p2", bufs=1)
                nc.tensor.matmul(out=p2[:], lhsT=w2_r[:], rhs=h1[:],
                                 start=True, stop=True)
                h2 = pool.tile([128, N], dt.float32r, tag="h2", bufs=3)
                nc.scalar.activation(h2[:], p2[:],
                                     mybir.ActivationFunctionType.Relu,
                                     bias=b2_s[:])
                p3 = ppool.tile([4, N], dt.float32, tag="p3", bufs=1)
                nc.tensor.matmul(out=p3[:], lhsT=w3_r[:], rhs=h2[:],
                                 start=True, stop=True)
                out4 = pool.tile([4, N], dt.float32, tag="out4", bufs=4)
                nc.scalar.activation(out4[:], p3[:],
                                     mybir.ActivationFunctionType.Relu,
                                     bias=b3_s[:])
                nc.scalar.add(out=out4[:], in_=out4[:], add=sub4_s[:])
                nc.vector.tensor_tensor(out=out4[:], in0=out4[:],
                                        in1=rhs[0:4, :].bitcast(dt.float32),
                                        op=mybir.AluOpType.mult)
                nc.scalar.add(out=out4[:], in_=out4[:], add=add4_s[:])
                # transpose each [4,128] block to pixel-major [128,4]
                pot = ppool.tile([128, 4 * gb], dt.float32, tag="pot", bufs=2)
                for j in range(gb):
                    nc.tensor.transpose(out=pot[:, 4 * j:4 * (j + 1)],
                                        in_=out4[:, P * j:P * (j + 1)],
                                        identity=ident[:4, :4])
                stage = pool.tile([128, 4 * gb], dt.float32, tag="stage",
                                  bufs=4)
                nc.vector.tensor_copy(out=stage[:], in_=pot[:])
                # partition p rows (p*cols + gb*g .. + gb) = 64B contiguous
                osl = bass.AP(out.ap().tensor, (gb * g) * 4,
                              [[cols * 4, P], [1, 4 * gb]])
                nc.sync.dma_start(out=osl, in_=stage[:])
    nc.compile()
    return nc


def _make_callable(nc, n_cores):
    install_neuronx_cc_hook()
    partition_name = nc.partition_id_tensor.name if nc.partition_id_tensor else None
    in_names, out_names, out_avals, zero_outs = [], [], [], []
    for alloc in nc.m.functions[0].allocations:
        if not isinstance(alloc, mybir.MemoryLocationSet):
            continue
        name = alloc.memorylocations[0].name
        if alloc.kind == "ExternalInput":
            if name != partition_name:
                in_names.append(name)
        elif alloc.kind == "ExternalOutput":
            out_names.append(name)
            shape = tuple(alloc.tensor_shape)
            dtype = mybir.dt.np(alloc.dtype)
            out_avals.append(jax.core.ShapedArray(shape, dtype))
            zero_outs.append(np.zeros(shape, dtype))
    n_params = len(in_names)
    all_in_names = list(in_names) + list(out_names)
    if partition_name is not None:
        all_in_names.append(partition_name)

    def _body(*args):
        operands = list(args)
        if partition_name is not None:
            operands.append(partition_id_tensor())
        outs = _bass_exec_p.bind(
            *operands,
            out_avals=tuple(out_avals),
            in_names=tuple(all_in_names),
            out_names=tuple(out_names),
            lowering_input_output_aliases=(),
            sim_require_finite=True,
            sim_require_nnan=True,
            nc=nc,
        )
        return tuple(outs)

    devices = jax.devices()[:n_cores]
    mesh = Mesh(np.asarray(devices), ("core",))
    in_specs = (PartitionSpec("core"),) * (n_params + len(out_names))
    out_specs = (PartitionSpec("core"),) * len(out_names)
    fn = jax.jit(
        shard_map(_body, mesh=mesh, in_specs=in_specs, out_specs=out_specs,
                  check_rep=False),
        keep_unused=True,
    )
    return fn, in_names, out_names, zero_outs, mesh


def _prep_in_maps(pix_to_face, bary_coords, faces, feature,
                  W1, b1, W2, b2, W3, b3):
    # host-side O(F) table join: per-face 9-vector of its 3 vertices' features
    gtab = np.ascontiguousarray(
        feature.astype(np.float32)[faces.astype(np.int64)].reshape(F, 9))
    w1e = np.zeros((13, 128), np.float32)
    # stanza rows 0-3: mask (zero weight); rows 4-12: t, row 4+3v+c = b_v*G[v,c]
    w1e[4:13] = np.tile(W1.astype(np.float32) / 3.0, (3, 1))
    w3p = np.concatenate([W3.astype(np.float32),
                          np.zeros((128, 1), np.float32)], axis=1)
    b3p = np.concatenate([b3.astype(np.float32),
                          np.zeros(1, np.float32)]).reshape(4, 1)
    shared = {
        "gtab": gtab,
        "w1e": w1e,
        "b1c": b1.reshape(128, 1).astype(np.float32),
        "w2": W2.astype(np.float32),
        "b2c": b2.reshape(128, 1).astype(np.float32),
        "w3": w3p,
        "b3c": b3p,
        "sub4": np.array([-2, -2, -2, 1], np.float32).reshape(4, 1),
        "add4": np.array([1, 1, 1, 0], np.float32).reshape(4, 1),
    }
    in_maps = []
    for b in range(B):
        m = dict(shared)
        m["pf"] = np.ascontiguousarray(
            pix_to_face[b, :, :, 0].reshape(P, COLS).astype(np.int32))
        m["bary"] = np.ascontiguousarray(
            bary_coords[b, :, :, 0, :].reshape(P, COLS * 3).astype(np.float32))
        in_maps.append(m)
    return in_maps


def _get_runner():
    if "runner" not in _CACHE:
        nc = _build_kernel()
        fn, in_names, out_names, zero_outs, mesh = _make_callable(nc, B)
        _CACHE["runner"] = (fn, in_names, out_names, zero_outs, mesh)
    return _CACHE["runner"]


def prepare(in_maps):
    """Device-put the concatenated inputs once; returns an opaque handle."""
    fn, in_names, out_names, zero_outs, mesh = _get_runner()
    concat_in = [np.concatenate([np.asarray(m[name]) for m in in_maps], axis=0)
                 for name in in_names]
    concat_zeros = [np.zeros((B * z.shape[0], *z.shape[1:]), z.dtype)
                    for z in zero_outs]
    sh = NamedSharding(mesh, PartitionSpec("core"))
    args = [jax.device_put(a, sh) for a in concat_in + concat_zeros]
    return (fn, out_names, args)


def execute_nofetch(handle):
    """Run on device-resident inputs, block, but do not fetch outputs."""
    fn, out_names, args = handle
    outs = fn(*args)
    jax.block_until_ready(outs)
    return outs


def execute(handle):
    """Run the kernel on device-resident inputs; returns per-core out dicts."""
    fn, out_names, args = handle
    outs = fn(*args)
    jax.block_until_ready(outs)
    res = []
    for c in range(B):
        d = {}
        for i, name in enumerate(out_names):
            a = np.asarray(outs[i])
            per = a.shape[0] // B
            d[name] = a[c * per:(c + 1) * per]
        res.append(d)
    return res


def run_on_device(in_maps):
    """Execute the compiled kernel on the 8 cores; returns per-core out dict."""
    return execute(prepare(in_maps))


def kernel(pix_to_face, bary_coords, faces, feature,
           W1, b1, W2, b2, W3, b3):
    pix_to_face = np.asarray(pix_to_face)
    bary_coords = np.asarray(bary_coords)
    faces = np.asarray(faces)
    feature = np.asarray(feature)
    in_maps = _prep_in_maps(pix_to_face, bary_coords, faces, feature,
                            np.asarray(W1), np.asarray(b1), np.asarray(W2),
                            np.asarray(b2), np.asarray(W3), np.asarray(b3))
    results = run_on_device(in_maps)
    out = np.stack([results[b]["out"].reshape(H, W, 4) for b in range(B)],
                   axis=0)
    return out
